# revision 2
# baseline (speedup 1.0000x reference)
"""RGCN (2-layer, basis decomposition) link-predict encoder on 8 Trainium2 cores.

E8 design — descriptor-count-minimized two-phase kernel.

The Q7/SWDGE descriptor generation rate (~7ns/row, engine-serial) bounds any
gather-based kernel, so the design minimizes descriptor rows:
  - Edges sharded by dst block; per-core edges sorted by (src-half, etype,
    dst-window), with each (group, window) run padded to a multiple of 8.
  - Layer-1 phase-1 needs NO gather at all: the host pre-gathers and
    pre-transposes h0[src] into x1T (streamed contiguously), and sends
    h0blk^T for the self-loop. Layer-2 phase-1 is a batched dma_gather of
    h1[src]^T (transpose=True).
  - Phase 1 per tile: one matmul with W_etype -> PSUM [e,d]; norm-scaled
    copy (alternating ScalarE/VectorE) -> bf16 messages -> DRAM.
  - Phase 2 gathers messages in OCTETS: one 2KB descriptor = 8 consecutive
    same-window messages (8x fewer rows). Blocks of 128 descs are
    window-pure (window desc counts padded to 128). Per block: one wide
    indicator build (VectorE, [128, 8*128]) + 8 indicator matmuls
    accumulating out[slot, d] in PSUM (natural layout, no transposes).
  - Self-loop accumulates into the same PSUM (lhsT = h_blk^T tile).
    Epilogue: bias add (VectorE) + ReLU (ScalarE, layer 1) per window.
  - One AllGather of h1 (bf16, Shared scratchpad) between layers.
"""

import os
import sys
import numpy as np

for _p in ("/opt/trn_rl_repo", "/root/.axon_site/_ro/trn_rl_repo"):
    if os.path.isdir(_p) and _p not in sys.path:
        sys.path.append(_p)

import ml_dtypes
import concourse.bass as bass
import concourse.mybir as mybir
import concourse.tile as tile
import concourse.bacc as bacc
from concourse.bass_utils import run_bass_kernel_spmd

P = 128
GB = 32           # phase-1 tiles per stream/gather call
DPC = 1024        # phase-2 descs per gather call (= 8 blocks)
WB = 8            # windows per epilogue write block


def _ceil_div(a, b):
    return (a + b - 1) // b


def _wrap_idx16(flat):
    """[n*128] ints -> [128, n*8] int16 wrapped: idx i at [i%16, i//16],
    replicated over the 8 16-partition stripes."""
    T = len(flat) // P
    a = np.asarray(flat, np.int16).reshape(T, 8, 16)
    a = np.ascontiguousarray(a.transpose(2, 0, 1).reshape(16, T * 8))
    return np.ascontiguousarray(np.tile(a, (8, 1)))


def _preprocess(src, dst, etype, norm, n_nodes, n_rels, n_cores):
    NB = n_nodes // n_cores
    NW = _ceil_div(NB, P)
    half = _ceil_div(n_nodes, 2) if n_nodes > 32767 else n_nodes
    n_halves = 2 if n_nodes > 32767 else 1
    NSG = n_halves * n_rels

    src = np.asarray(src, np.int64)
    dst = np.asarray(dst, np.int64)
    etype = np.asarray(etype, np.int64)
    norm = np.asarray(norm, np.float32).reshape(-1)

    # ---- per-core edge partition, sorted by (group=(half,etype), window) ----
    cores = []
    slots_g = np.zeros((n_cores, NSG), np.int64)   # octet-padded slots/group
    ndesc_w = np.zeros((n_cores, NW), np.int64)
    for c in range(n_cores):
        m = (dst // NB) == c
        es, ed, ee, en = src[m], dst[m], etype[m], norm[m]
        dl = ed - c * NB
        w = dl // P
        g = (es // half) * n_rels + ee
        order = np.lexsort((w, g))
        es, dl, en, w, g = es[order], dl[order], en[order], w[order], g[order]
        cnt_gw = np.zeros((NSG, NW), np.int64)
        np.add.at(cnt_gw, (g, w), 1)
        oct_gw = (cnt_gw + 7) // 8 * 8
        slots_g[c] = oct_gw.sum(1)
        ndesc_w[c] = (oct_gw // 8).sum(0)
        cores.append((es, dl, en, w, g, cnt_gw, oct_gw))

    T_g = [int(_ceil_div(int(slots_g[:, g].max()), P)) for g in range(NSG)]
    base_g = np.concatenate([[0], np.cumsum(T_g)])
    T1 = int(base_g[-1])
    rel_of_tile = []
    for g in range(NSG):
        rel_of_tile += [g % n_rels] * T_g[g]

    ND_w = [int(_ceil_div(int(ndesc_w[:, w].max()), P)) * P for w in range(NW)]
    base_w = np.concatenate([[0], np.cumsum(ND_w)])
    ND = int(base_w[-1])
    NDp = _ceil_div(ND, DPC) * DPC

    # phase-1 calls: batches of <= GB tiles within one half
    p1_calls = []
    for hf in range(n_halves):
        t0 = int(base_g[hf * n_rels])
        t1 = int(base_g[(hf + 1) * n_rels])
        t = t0
        while t < t1:
            k = min(GB, t1 - t)
            p1_calls.append((hf, t, k))
            t += k

    per_core = []
    for c in range(n_cores):
        es, dl, en, w, g, cnt_gw, oct_gw = cores[c]
        p1_idx = np.zeros(T1 * P, np.int64)
        p1_norm = np.zeros(T1 * P, np.float32)
        desc_addr = np.zeros(NDp, np.int64)
        desc_sv = np.full((NDp, 8), -1.0, np.float32)

        # slot assignment: per group, windows in order, runs octet-padded
        gw_base = np.zeros((NSG, NW), np.int64)
        for gi in range(NSG):
            off = int(base_g[gi]) * P
            for wi in range(NW):
                gw_base[gi, wi] = off
                off += int(oct_gw[gi, wi])
        pos_in_gw = np.zeros((NSG, NW), np.int64)
        slots = np.empty(len(es), np.int64)
        for i in range(len(es)):
            gi, wi = g[i], w[i]
            slots[i] = gw_base[gi, wi] + pos_in_gw[gi, wi]
            pos_in_gw[gi, wi] += 1
        p1_idx[slots] = es - (g // n_rels) * half
        p1_norm[slots] = en

        # desc lists per window
        dpos = [int(base_w[wi]) for wi in range(NW)]
        sv_of_slot = np.full(T1 * P, -1.0, np.float32)
        sv_of_slot[slots] = (dl - w * P).astype(np.float32)
        for gi in range(NSG):
            for wi in range(NW):
                n_oct = int(oct_gw[gi, wi]) // 8
                if n_oct == 0:
                    continue
                s0 = gw_base[gi, wi]
                for o in range(n_oct):
                    d_i = dpos[wi]
                    dpos[wi] += 1
                    a = s0 + o * 8
                    desc_addr[d_i] = a // 8
                    desc_sv[d_i] = sv_of_slot[a: a + 8]

        nblk = NDp // P
        p2s = np.ascontiguousarray(
            desc_sv.reshape(nblk, P, 8).transpose(1, 0, 2).reshape(P, nblk * 8)
        ).astype(ml_dtypes.bfloat16)

        per_core.append(dict(
            p1i=_wrap_idx16(p1_idx),
            p1n=np.ascontiguousarray(p1_norm.reshape(T1, P).T),
            p2i=_wrap_idx16(desc_addr),
            p2s=p2s,
            slots=slots, es=es,
        ))

    selfi = _wrap_idx16(np.arange(NW * P) % NB)

    struct = dict(
        NB=NB, NW=NW, T1=T1, n_halves=n_halves, half=half,
        rel_of_tile=rel_of_tile, p1_calls=p1_calls,
        base_w=[int(x) for x in base_w], ND=ND, NDp=NDp,
        n_rels=n_rels, n_cores=n_cores,
    )
    return struct, per_core, selfi


def _build_program(struct, n_nodes, d):
    NB, NW, T1 = struct["NB"], struct["NW"], struct["T1"]
    rel_of_tile = struct["rel_of_tile"]
    p1_calls = struct["p1_calls"]
    base_w, NDp = struct["base_w"], struct["NDp"]
    n_rels = struct["n_rels"]
    n_cores = struct["n_cores"]
    half = struct["half"]
    NGW = n_rels + 1
    f32, bf16, i16 = mybir.dt.float32, mybir.dt.bfloat16, mybir.dt.int16
    i32 = mybir.dt.int32
    Act = mybir.ActivationFunctionType
    NOCOLL = bool(int(os.environ.get('KE8_NOCOLL', '0')))

    nc = bacc.Bacc("TRN2", target_bir_lowering=False, debug=False,
                   num_devices=n_cores)

    x1T = nc.dram_tensor("x1T", [P, T1 * P], bf16, kind="ExternalInput")
    h0bT = nc.dram_tensor("h0bT", [P, NW * P], bf16, kind="ExternalInput")
    w1 = nc.dram_tensor("w1", [d, NGW * d], bf16, kind="ExternalInput")
    w2 = nc.dram_tensor("w2", [d, NGW * d], bf16, kind="ExternalInput")
    b1 = nc.dram_tensor("b1", [P, d], f32, kind="ExternalInput")
    b2 = nc.dram_tensor("b2", [P, d], f32, kind="ExternalInput")
    p1i = nc.dram_tensor("p1i", [P, T1 * 8], i16, kind="ExternalInput")
    p1n = nc.dram_tensor("p1n", [P, T1], f32, kind="ExternalInput")
    p2i = nc.dram_tensor("p2i", [P, NDp // 16], i16, kind="ExternalInput")
    p2s = nc.dram_tensor("p2s", [P, NDp // P * 8], bf16, kind="ExternalInput")
    sfi = nc.dram_tensor("sfi", [P, NW * 8], i16, kind="ExternalInput")
    out = nc.dram_tensor("out", [NB, d], f32, kind="ExternalOutput")

    msgs = nc.dram_tensor("msgs", [T1 * P, d], bf16)
    h1blk = nc.dram_tensor("h1blk", [NB, d], bf16)
    h1full = nc.dram_tensor("h1full", [n_cores * NB, d], bf16,
                            addr_space="Shared")

    with tile.TileContext(nc) as tc:
        with (
            tc.tile_pool(name="cst", bufs=1) as cst,
            tc.tile_pool(name="g1p", bufs=3) as g1p,
            tc.tile_pool(name="mbp", bufs=3) as mbp,
            tc.tile_pool(name="g2p", bufs=2) as g2p,
            tc.tile_pool(name="sfp", bufs=2) as sfp,
            tc.tile_pool(name="indp", bufs=4) as indp,
            tc.tile_pool(name="obp", bufs=2) as obp,
            tc.tile_pool(name="hbp", bufs=2) as hbp,
            tc.tile_pool(name="ps_m", bufs=3, space="PSUM") as ps_m,
            tc.tile_pool(name="ps_o", bufs=2, space="PSUM") as ps_o,
        ):
            p1i_sb = cst.tile([P, T1 * 8], i16)
            nc.sync.dma_start(p1i_sb[:], p1i[:, :])
            p1n_sb = cst.tile([P, T1], f32)
            nc.sync.dma_start(p1n_sb[:], p1n[:, :])
            p2i_sb = cst.tile([P, NDp // 16], i16)
            nc.sync.dma_start(p2i_sb[:], p2i[:, :])
            p2s_sb = cst.tile([P, NDp // P * 8], bf16)
            nc.sync.dma_start(p2s_sb[:], p2s[:, :])
            sfi_sb = cst.tile([P, NW * 8], i16)
            nc.sync.dma_start(sfi_sb[:], sfi[:, :])
            w1_sb = cst.tile([P, NGW * d], bf16)
            nc.sync.dma_start(w1_sb[:], w1[:, :])
            w2_sb = cst.tile([P, NGW * d], bf16)
            nc.sync.dma_start(w2_sb[:], w2[:, :])
            b1_sb = cst.tile([P, d], f32)
            nc.sync.dma_start(b1_sb[:], b1[:, :])
            b2_sb = cst.tile([P, d], f32)
            nc.sync.dma_start(b2_sb[:], b2[:, :])
            h0bT_sb = cst.tile([P, NW * P], bf16)
            nc.sync.dma_start(h0bT_sb[:], h0bT[:, :])
            iota32 = cst.tile([P, 8 * P], i32)
            nc.gpsimd.iota(iota32[:], pattern=[[0, 8], [1, P]], base=0,
                           channel_multiplier=0)
            iotaS = cst.tile([P, 8 * P], bf16)
            nc.vector.tensor_copy(iotaS[:], iota32[:])

            def block_write(dram, sb_ap, w0, rows):
                full = (rows // P) * P
                if full:
                    nc.sync.dma_start(
                        dram[w0 * P: w0 * P + full, :]
                        .rearrange("(t p) d -> p t d", p=P),
                        sb_ap[:, : full // P * d].rearrange(
                            "p (t d) -> p t d", d=d),
                    )
                rem = rows - full
                if rem:
                    nc.sync.dma_start(
                        dram[w0 * P + full: w0 * P + rows, :],
                        sb_ap[:rem, full // P * d: (full // P + 1) * d],
                    )

            def layer(lnum, h_src, w_sb, bias_sb, out_dram):
                relu = lnum == 1
                # ---- self-loop h_blk^T ----
                if lnum == 1:
                    sxt = h0bT_sb
                else:
                    sxt = sfp.tile([P, NW * P], bf16, tag="sxt")
                    for s0 in range(0, NW, GB):
                        kt = min(GB, NW - s0)
                        nc.gpsimd.dma_gather(
                            out_ap=sxt[:, s0 * P: (s0 + kt) * P].rearrange(
                                "p (o ni) -> p o ni", o=1),
                            in_ap=h1blk[:, :],
                            idxs_ap=sfi_sb[:, s0 * 8: (s0 + kt) * 8],
                            num_idxs=kt * P, num_idxs_reg=kt * P,
                            elem_size=d, transpose=True, single_packet=False,
                        )

                # ---- phase 1: transform, messages to DRAM ----
                for (hf, t0, kt) in p1_calls:
                    ni = kt * P
                    xt = g1p.tile([P, GB * P], bf16, tag="g1")
                    if lnum == 1:
                        nc.sync.dma_start(
                            xt[:, :ni], x1T[:, t0 * P: (t0 + kt) * P])
                    else:
                        nc.gpsimd.dma_gather(
                            out_ap=xt[:, :ni].rearrange(
                                "p (o ni) -> p o ni", o=1),
                            in_ap=h_src[hf * half:
                                        min((hf + 1) * half, n_nodes), :],
                            idxs_ap=p1i_sb[:, t0 * 8: (t0 + kt) * 8],
                            num_idxs=ni, num_idxs_reg=ni,
                            elem_size=d, transpose=True, single_packet=False,
                        )
                    mb = mbp.tile([P, GB * d], bf16, tag="mb")
                    for k in range(kt):
                        t = t0 + k
                        r = rel_of_tile[t]
                        m_ps = ps_m.tile([P, d], f32, tag="mp", space="PSUM")
                        nc.tensor.matmul(
                            out=m_ps[:],
                            lhsT=xt[:, k * P: (k + 1) * P],
                            rhs=w_sb[:, r * d: (r + 1) * d],
                            start=True, stop=True,
                        )
                        if t % 2 == 0:
                            nc.scalar.activation(
                                mb[:, k * d: (k + 1) * d], m_ps[:],
                                Act.Copy, scale=p1n_sb[:, t: t + 1])
                        else:
                            nc.vector.tensor_tensor(
                                out=mb[:, k * d: (k + 1) * d], in0=m_ps[:],
                                in1=p1n_sb[:, t: t + 1].to_broadcast([P, d]),
                                op=mybir.AluOpType.mult)
                    nc.sync.dma_start(
                        msgs[t0 * P: (t0 + kt) * P, :].rearrange(
                            "(t p) d -> p t d", p=P),
                        mb[:, : kt * d].rearrange("p (t d) -> p t d", d=d),
                    )

                # ---- phase 2: octet gather + indicator matmuls ----
                g2_of_call = {}

                def ensure_call(q):
                    if q in g2_of_call:
                        return g2_of_call[q]
                    g2 = g2p.tile([P, DPC * 8], bf16, tag="g2")
                    nc.gpsimd.dma_gather(
                        out_ap=g2[:].rearrange("p (b e) -> p b e", e=8 * d),
                        in_ap=msgs[:, :].rearrange(
                            "(o e) d -> o (e d)", e=8),
                        idxs_ap=p2i_sb[:, q * (DPC // 16):
                                       (q + 1) * (DPC // 16)],
                        num_idxs=DPC, num_idxs_reg=DPC,
                        elem_size=8 * d, transpose=False, single_packet=False,
                    )
                    g2_of_call[q] = g2
                    return g2

                hb = None
                ob = None
                hb_w0 = 0
                for w in range(NW):
                    o_ps = ps_o.tile([P, d], f32, tag="op", space="PSUM")
                    nc.tensor.matmul(
                        out=o_ps[:],
                        lhsT=sxt[:, w * P: (w + 1) * P],
                        rhs=w_sb[:, n_rels * d: (n_rels + 1) * d],
                        start=True, stop=False,
                    )
                    kb0 = base_w[w] // P
                    kb1 = base_w[w + 1] // P
                    for kb in range(kb0, kb1):
                        g2 = ensure_call(kb // (DPC // P))
                        koff = kb % (DPC // P)
                        ind = indp.tile([P, 8 * P], bf16, tag="ind")
                        nc.vector.tensor_tensor(
                            out=ind[:].rearrange("p (m s) -> p m s", s=P),
                            in0=iotaS[:].rearrange("p (m s) -> p m s", s=P),
                            in1=p2s_sb[:, kb * 8: (kb + 1) * 8].rearrange(
                                "p (m x) -> p m x", x=1
                            ).to_broadcast([P, 8, P]),
                            op=mybir.AluOpType.is_equal)
                        for mI in range(8):
                            nc.tensor.matmul(
                                out=o_ps[:],
                                lhsT=ind[:, mI * P: (mI + 1) * P],
                                rhs=g2[:, koff * 8 * d + mI * d:
                                       koff * 8 * d + (mI + 1) * d],
                                start=False,
                                stop=(kb == kb1 - 1) and (mI == 7),
                            )
                    # epilogue for window w
                    if hb is None:
                        hb = obp.tile([P, WB * d], f32, tag="ob")
                        if relu:
                            hbB = hbp.tile([P, WB * d], bf16, tag="hbB")
                        hb_w0 = w
                    j = w - hb_w0
                    nc.vector.tensor_add(
                        hb[:, j * d: (j + 1) * d], o_ps[:], bias_sb[:])
                    if relu:
                        nc.scalar.activation(
                            hbB[:, j * d: (j + 1) * d],
                            hb[:, j * d: (j + 1) * d], Act.Relu)
                    if j + 1 == WB or w == NW - 1:
                        rows = min((j + 1) * P, NB - hb_w0 * P)
                        block_write(out_dram, hbB[:] if relu else hb[:],
                                    hb_w0, rows)
                        hb = None

            layer(1, None, w1_sb, b1_sb, h1blk)
            if n_cores > 1 and not NOCOLL:
                nc.gpsimd.collective_compute(
                    "AllGather", mybir.AluOpType.bypass,
                    replica_groups=[list(range(n_cores))],
                    ins=[h1blk.ap().opt()], outs=[h1full.ap().opt()],
                )
                l2_src = h1full
            else:
                l2_src = h1blk
            layer(2, l2_src, w2_sb, b2_sb, out)

    nc.finalize()
    return nc


_CACHE = {}


def _get_program(struct, n_nodes, d):
    key = (n_nodes, d, struct["T1"], struct["NDp"],
           tuple(struct["rel_of_tile"]), tuple(struct["base_w"]),
           struct["n_cores"])
    if key not in _CACHE:
        _CACHE[key] = _build_program(struct, n_nodes, d)
    return _CACHE[key]


def prepare(h_ids, src, dst, etype, norm, embedding,
            w_comp1, bases1, loop_w1, bias1,
            w_comp2, bases2, loop_w2, bias2, n_cores=8):
    h_ids = np.asarray(h_ids).astype(np.int64)
    src = np.asarray(src).astype(np.int64)
    dst = np.asarray(dst).astype(np.int64)
    etype = np.asarray(etype).astype(np.int64)
    norm = np.asarray(norm, dtype=np.float32)
    embedding = np.asarray(embedding, dtype=np.float32)
    n_nodes, d = embedding.shape
    n_rels = np.asarray(w_comp1).shape[0]
    NB = n_nodes // n_cores

    W1 = np.einsum("rb,bio->rio", np.asarray(w_comp1, np.float64),
                   np.asarray(bases1, np.float64)).astype(np.float32)
    W2 = np.einsum("rb,bio->rio", np.asarray(w_comp2, np.float64),
                   np.asarray(bases2, np.float64)).astype(np.float32)
    W1 = np.concatenate([W1, np.asarray(loop_w1, np.float32)[None]], 0)
    W2 = np.concatenate([W2, np.asarray(loop_w2, np.float32)[None]], 0)
    w1_dev = np.ascontiguousarray(
        np.transpose(W1, (1, 0, 2)).reshape(d, (n_rels + 1) * d)
    ).astype(ml_dtypes.bfloat16)
    w2_dev = np.ascontiguousarray(
        np.transpose(W2, (1, 0, 2)).reshape(d, (n_rels + 1) * d)
    ).astype(ml_dtypes.bfloat16)
    b1_dev = np.ascontiguousarray(
        np.broadcast_to(np.asarray(bias1, np.float32)[None, :], (P, d)))
    b2_dev = np.ascontiguousarray(
        np.broadcast_to(np.asarray(bias2, np.float32)[None, :], (P, d)))
    h0 = embedding[h_ids].astype(ml_dtypes.bfloat16)

    struct, per_core, selfi = _preprocess(
        src, dst, etype, norm, n_nodes, n_rels, n_cores)
    T1 = struct["T1"]

    in_maps = []
    for c in range(n_cores):
        pc = per_core[c]
        # host-side pre-gather + per-tile transpose for layer-1 phase-1
        x1 = np.zeros((T1 * P, d), ml_dtypes.bfloat16)
        x1[pc["slots"]] = h0[pc["es"]]
        x1T = np.ascontiguousarray(
            x1.reshape(T1, P, d).transpose(0, 2, 1)      # [T1, d, P]
            .transpose(1, 0, 2).reshape(d, T1 * P))      # [d, T1*P]
        h0bT = np.zeros((P, struct["NW"] * P), ml_dtypes.bfloat16)
        blk = h0[c * NB:(c + 1) * NB]
        idx = np.arange(struct["NW"] * P) % NB
        h0bT[:, :] = blk[idx].T
        in_maps.append({
            "x1T": x1T, "h0bT": np.ascontiguousarray(h0bT),
            "w1": w1_dev, "w2": w2_dev, "b1": b1_dev, "b2": b2_dev,
            "p1i": pc["p1i"], "p1n": pc["p1n"],
            "p2i": pc["p2i"], "p2s": pc["p2s"], "sfi": selfi,
        })
    return struct, in_maps, n_nodes, d


def run(h_ids, src, dst, etype, norm, embedding,
        w_comp1, bases1, loop_w1, bias1,
        w_comp2, bases2, loop_w2, bias2,
        n_cores=8, trace=False):
    struct, in_maps, n_nodes, d = prepare(
        h_ids, src, dst, etype, norm, embedding,
        w_comp1, bases1, loop_w1, bias1,
        w_comp2, bases2, loop_w2, bias2, n_cores)
    nc = _get_program(struct, n_nodes, d)
    res = run_bass_kernel_spmd(
        nc, in_maps, core_ids=list(range(n_cores)), trace=trace)
    blocks = [res.results[c]["out"] for c in range(n_cores)]
    full = np.concatenate(blocks, 0)[:n_nodes]
    if trace:
        return full, res
    return full


def kernel(h_ids, src, dst, etype, norm, embedding,
           w_comp1, bases1, loop_w1, bias1,
           w_comp2, bases2, loop_w2, bias2):
    return run(h_ids, src, dst, etype, norm, embedding,
               w_comp1, bases1, loop_w1, bias1,
               w_comp2, bases2, loop_w2, bias2)
